# revision 2
# baseline (speedup 1.0000x reference)
"""CliffordLayerNorm Trainium2 kernel.

x: [16, 4096, 1024] fp32. Each row's 1024 features = 4 blocks of 256
multivector components; components are grouped into 9 grades by popcount of
their index within the block.  Per (token, block, grade): mean/var, then
out = (x - mean) * w[g] * rsqrt(var + eps) + b[g].

Strategy (per NeuronCore, data-parallel over tokens across 8 cores):
  1. DMA in token-major tiles [128 tok, 1024 feat].
  2. PE-transpose each 128x128 chunk into PSUM (feature-major).
  3. ACT copies PSUM -> SBUF (x_T) and squares PSUM -> SBUF (sq_T).
  4. PE matmuls against a grade-membership matrix (entries 1/count) give
     per-(block,grade) mean and mean-of-squares directly: PSUM [72, T].
  5. Small DVE/ACT/GPSIMD ops produce rstd and (b/w - mean*rstd) stats.
  6. PE scatter-matmuls (stats as stationary, w-scaled grade indicator as
     moving) expand stats back to per-element scale A and shift B in
     token-major layout.
  7. DVE: out = x * A + B, DMA out.
"""

import os
import sys

if "/opt/trn_rl_repo" not in sys.path:
    sys.path.insert(0, "/opt/trn_rl_repo")

import numpy as np

BLOCK_BITS = 8
MV = 256
NG = 9
NB = 4
D = 1024
EPS = 1e-5
N_CORES = 8
TOTAL_TOKENS = 16 * 4096
TOK_PER_CORE = TOTAL_TOKENS // N_CORES  # 8192

GROUP_T = 256          # tokens per stats group
TILE_T = 128           # tokens per tile (partition dim)

# Matmul operand dtype: float32r runs at 1 cycle/row (vs 4 for float32) on
# the PE at N>=256; accumulation stays fp32 in PSUM.
USE_F32R = True


def _grade(m):
    return bin(m).count("1")


def _build_consts():
    import math
    counts = np.array([math.comb(8, g) for g in range(NG)], dtype=np.float32)

    # G_mean[h][i, b*9+g] = 1/count_g  for chunk h (features 128h..128h+127)
    gmean = np.zeros((8, 128, 36), dtype=np.float32)
    for h in range(8):
        b = h // 2
        for i in range(128):
            m = (h % 2) * 128 + i
            g = _grade(m)
            gmean[h, i, b * 9 + g] = 1.0 / counts[g]

    # G01[b*9+g, c] = 1 if feature c belongs to (block b, grade g)
    g01 = np.zeros((36, D), dtype=np.float32)
    for c in range(D):
        b = c // MV
        g = _grade(c % MV)
        g01[b * 9 + g, c] = 1.0

    # rstd mask: count-1 grades (0 and 8) have centered value exactly 0 in
    # the reference, so any scale works -- force rstd=0 there to avoid
    # amplifying f32r rounding by rsqrt(eps).
    mask = np.ones((36, 1), dtype=np.float32)
    for b in range(NB):
        mask[b * 9 + 0, 0] = 0.0
        mask[b * 9 + 8, 0] = 0.0
    return gmean, g01, mask


def build_nc(tok_per_core=TOK_PER_CORE, use_f32r=USE_F32R, loop_reps=1):
    import concourse.bass as bass
    import concourse.tile as tile
    from concourse import bacc, mybir

    f32 = mybir.dt.float32
    f32r = mybir.dt.float32r
    AF = mybir.ActivationFunctionType
    ALU = mybir.AluOpType

    fmm = f32r if use_f32r else f32
    fst = mybir.dt.bfloat16 if use_f32r else f32   # stats-matmul operand dtype

    gmean_np, g01_np, mask_np = _build_consts()
    n_groups = tok_per_core // GROUP_T
    assert tok_per_core % GROUP_T == 0

    nc = bacc.Bacc()
    x_d = nc.dram_tensor("x", [tok_per_core, D], f32, kind="ExternalInput")
    w_d = nc.dram_tensor("weight", [NG], f32, kind="ExternalInput")
    b_d = nc.dram_tensor("bias", [NG], f32, kind="ExternalInput")
    out_d = nc.dram_tensor("out", [tok_per_core, D], f32, kind="ExternalOutput")

    gmean_dram = nc.inline_tensor(gmean_np, name="gmean_const")
    g01_dram = nc.inline_tensor(g01_np, name="g01_const")
    ident_dram = nc.inline_tensor(np.eye(128, dtype=np.float32), name="ident_const")
    mask_dram = nc.inline_tensor(mask_np, name="mask_const")

    from contextlib import ExitStack

    with tile.TileContext(nc) as tc, ExitStack() as ctx:
        consts = ctx.enter_context(tc.tile_pool(name="consts", bufs=1))
        xg_pool = ctx.enter_context(tc.tile_pool(name="xg", bufs=10))
        xt_pool = ctx.enter_context(tc.tile_pool(name="xt", bufs=4))
        sqt_pool = ctx.enter_context(tc.tile_pool(name="sqt", bufs=4))
        tmp_pool = ctx.enter_context(tc.tile_pool(name="tmp", bufs=6))
        small_pool = ctx.enter_context(tc.tile_pool(name="small", bufs=4))
        ps_xt = ctx.enter_context(tc.tile_pool(name="ps_xt", bufs=2, space="PSUM"))
        ps_stats = ctx.enter_context(tc.tile_pool(name="ps_st", bufs=2, space="PSUM"))
        ps_a = ctx.enter_context(tc.tile_pool(name="ps_a", bufs=2, space="PSUM"))
        ps_b = ctx.enter_context(tc.tile_pool(name="ps_b", bufs=2, space="PSUM"))

        # ---- constants into SBUF ----
        # All const DMAs go through gpsimd (SWDGE, single queue -> single
        # semaphore) so downstream compute needs at most one new wait.
        ident = consts.tile([128, 128], f32)
        nc.gpsimd.dma_start(out=ident, in_=ident_dram[:])

        gmean_f = consts.tile([128, 8, 36], f32)
        nc.gpsimd.dma_start(out=gmean_f, in_=gmean_dram[:].rearrange("h p c -> p h c"))

        g01_sb = consts.tile([36, D], f32)
        nc.gpsimd.dma_start(out=g01_sb, in_=g01_dram[:])

        # weight/bias broadcast to 36 partitions: partition p = b*9+g reads w[g]
        w36 = consts.tile([36, 1], f32)
        b36 = consts.tile([36, 1], f32)
        wap = w_d[:]
        bap = b_d[:]
        nc.gpsimd.dma_start(
            out=w36, in_=bass.AP(tensor=wap.tensor, offset=wap.offset,
                                 ap=[[0, NB]] + list(wap.ap)))
        nc.gpsimd.dma_start(
            out=b36, in_=bass.AP(tensor=bap.tensor, offset=bap.offset,
                                 ap=[[0, NB]] + list(bap.ap)))

        mask36 = consts.tile([36, 1], f32)
        nc.gpsimd.dma_start(out=mask36, in_=mask_dram[:])
        # eps + 1e38*(1-mask): count-1 grades get a huge bias so the fused
        # abs-rsqrt returns ~1e-19 (i.e. rstd ~= 0) for them
        eps36 = consts.tile([36, 1], f32)
        nc.vector.tensor_scalar(
            out=eps36, in0=mask36, scalar1=-1e38, scalar2=1e38 + EPS,
            op0=ALU.mult, op1=ALU.add)
        gmean_sb = consts.tile([128, 8, 36], fst)
        nc.vector.tensor_scalar_mul(gmean_sb, gmean_f, 1.0)
        rw36 = consts.tile([36, 1], f32)
        nc.vector.reciprocal(rw36, w36)
        # GA[bg, c] = w[g(c)] * indicator; ga_mask additionally zeroes
        # count-1 grades (their centered value is exactly 0 in the reference)
        ga_sb = consts.tile([36, D], fmm)
        nc.vector.tensor_scalar_mul(ga_sb, g01_sb, w36)
        w36m = consts.tile([36, 1], f32)
        nc.vector.tensor_scalar_mul(w36m, w36, mask36)
        ga_mask = consts.tile([36, D], fmm)
        nc.vector.tensor_scalar_mul(ga_mask, g01_sb, w36m)
        bw36 = consts.tile([36, 1], f32)   # b/w  (rw36 is 2 DVE insts old here)
        nc.vector.tensor_scalar_mul(bw36, b36, rw36)

        # ---- main loop ----
        rep_ctx = tc.For_i(0, loop_reps, 1) if loop_reps > 1 else None
        if rep_ctx is not None:
            rep_ctx.__enter__()
        for gi in range(n_groups):
            tok0 = gi * GROUP_T
            x_group = xg_pool.tile([128, 2, D], f32)
            nc.sync.dma_start(
                out=x_group,
                in_=x_d[tok0:tok0 + GROUP_T, :].rearrange("(j p) d -> p j d", p=128),
            )

            xT = xt_pool.tile([128, 8, GROUP_T], fst)
            sqT = sqt_pool.tile([128, 8, GROUP_T], fst)

            for j in range(2):
                for half in range(2):
                    xt_ps = ps_xt.tile([128, 512], f32)
                    for cc in range(4):
                        chunk = half * 4 + cc
                        nc.tensor.transpose(
                            xt_ps[:, cc * 128:(cc + 1) * 128],
                            x_group[:, j, chunk * 128:(chunk + 1) * 128],
                            ident,
                        )
                    src = xt_ps[:].rearrange("p (c t) -> p c t", c=4)
                    dst = (slice(None), slice(half * 4, (half + 1) * 4),
                           slice(j * 128, (j + 1) * 128))
                    nc.scalar.copy(out=xT[dst[0], dst[1], dst[2]], in_=src)
                    if j == 0 and half == 0:
                        # first unit's square on the idle GPSIMD (runs in
                        # parallel with the remaining ACT copies)
                        nc.gpsimd.tensor_tensor(
                            out=sqT[dst[0], dst[1], dst[2]],
                            in0=xT[dst[0], dst[1], dst[2]],
                            in1=xT[dst[0], dst[1], dst[2]], op=ALU.mult)
                    else:
                        nc.scalar.square(out=sqT[dst[0], dst[1], dst[2]],
                                         in_=xT[dst[0], dst[1], dst[2]])

            # stats: S12[:,0,:] = per-(block,grade) mean, S12[:,1,:] = mean of squares
            S12 = ps_stats.tile([36, 2, GROUP_T], f32)
            for h in range(8):
                nc.tensor.matmul(
                    S12[:, 0, :], gmean_sb[:, h, :], xT[:, h, :],
                    start=(h == 0), stop=(h == 7),
                )
            for h in range(8):
                nc.tensor.matmul(
                    S12[:, 1, :], gmean_sb[:, h, :], sqT[:, h, :],
                    start=(h == 0), stop=(h == 7),
                )

            stats_sb = small_pool.tile([36, 2, GROUP_T], f32)
            nc.scalar.copy(out=stats_sb, in_=S12)
            mean_sb = stats_sb[:, 0, :]
            mean2 = small_pool.tile([36, GROUP_T], f32)
            nc.gpsimd.tensor_tensor(out=mean2, in0=mean_sb, in1=mean_sb,
                                    op=ALU.mult)

            # var = ms - mean^2 (all SBUF, on the idle GPSIMD)
            var_t = small_pool.tile([36, GROUP_T], f32)
            nc.gpsimd.tensor_tensor(out=var_t, in0=stats_sb[:, 1, :],
                                    in1=mean2, op=ALU.subtract)
            # rstd = 1/sqrt(|var + eps|): abs also absorbs tiny negative var
            # from f32r rounding (count-1 grades are masked out anyway)
            rstd_t = small_pool.tile([36, GROUP_T], fmm)
            nc.scalar.activation(rstd_t, var_t, AF.Abs_reciprocal_sqrt,
                                 bias=eps36, scale=1.0)
            c_t = small_pool.tile([36, GROUP_T], f32)
            nc.gpsimd.tensor_tensor(out=c_t, in0=mean_sb, in1=rstd_t,
                                    op=ALU.mult)
            # c2n = b/w - mean*rstd
            c2n_t = small_pool.tile([36, GROUP_T], fmm)
            nc.gpsimd.tensor_scalar(
                out=c2n_t, in0=c_t, scalar1=bw36, scalar2=-1.0,
                op0=ALU.subtract, op1=ALU.mult,
            )

            for j in range(2):
                lhsA = rstd_t[:, j * 128:(j + 1) * 128]
                lhsB = c2n_t[:, j * 128:(j + 1) * 128]
                for half in range(2):
                    sl = slice(half * 512, (half + 1) * 512)
                    b_ps = ps_b.tile([128, 512], f32)
                    a_ps = ps_a.tile([128, 512], f32)
                    nc.tensor.matmul(b_ps, lhsB, ga_sb[:, sl])
                    nc.tensor.matmul(a_ps, lhsA, ga_mask[:, sl])
                    tmp = tmp_pool.tile([128, 512], f32)
                    nc.vector.scalar_tensor_tensor(
                        out=tmp, in0=x_group[:, j, sl], scalar=1.0, in1=a_ps,
                        op0=ALU.mult, op1=ALU.mult)
                    nc.vector.scalar_tensor_tensor(
                        out=x_group[:, j, sl], in0=tmp, scalar=1.0, in1=b_ps,
                        op0=ALU.mult, op1=ALU.add)

            nc.sync.dma_start(
                out=out_d[tok0:tok0 + GROUP_T, :].rearrange("(j p) d -> p j d", p=128),
                in_=x_group,
            )

        if rep_ctx is not None:
            rep_ctx.__exit__(None, None, None)

    nc.finalize()
    return nc


_NC_CACHE = {}


def _get_nc(tok_per_core=TOK_PER_CORE):
    key = (tok_per_core, USE_F32R)
    if key not in _NC_CACHE:
        _NC_CACHE[key] = build_nc(tok_per_core)
    return _NC_CACHE[key]


def kernel(x, weight, bias, _trace=False):
    x = np.ascontiguousarray(np.asarray(x, dtype=np.float32))
    weight = np.ascontiguousarray(np.asarray(weight, dtype=np.float32))
    bias = np.ascontiguousarray(np.asarray(bias, dtype=np.float32))
    orig_shape = x.shape
    xf = x.reshape(TOTAL_TOKENS, D)

    nc = _get_nc()
    from concourse.bass_utils import run_bass_kernel_spmd

    in_maps = [
        {
            "x": np.ascontiguousarray(xf[i * TOK_PER_CORE:(i + 1) * TOK_PER_CORE]),
            "weight": weight,
            "bias": bias,
        }
        for i in range(N_CORES)
    ]
    res = run_bass_kernel_spmd(nc, in_maps, core_ids=list(range(N_CORES)),
                               trace=_trace)
    out = np.concatenate([r["out"] for r in res.results], axis=0)
    if _trace:
        kernel.last_result = res
    return out.reshape(orig_shape)

